# revision 30
# baseline (speedup 1.0000x reference)
"""Trainium2 Bass kernel for the attention-LSTM captioning RNN.

Problem (per full batch): x(64,128,512), A(64,1024,4,4), Wx(512,4096),
Wh(1024,4096), Wattn(1024,4096), b(4096) -> h-sequence (64,128,1024).

Strategy: data-parallel over N across 8 cores (8 samples/core, weights
replicated).  Per core, recurrence in transposed ("a^T") layout: gates
live on 128 partitions (partition = gate-col % 128), batch (8) on the
free dim.  Weights are the stationary operands (FWL), h^T / x^T the
8-wide moving operands.

v4 pipeline summary:
  - all of a gate preactivation accumulates in one PSUM tile per 4-step
    group: x@Wx runs once per group with 32-wide moving (4 timesteps
    batched -> 4x fewer weight loads), bias via one indicator matmul per
    step, Wh and the attention folds accumulate per step, and the gate
    tanh (Scalar engine) reads PSUM directly with a merged strided AP
    (one activation per h-half covering all 4 quarters).
  - a single start=True per PSUM bank per group (start clears has_written
    for the whole bank); everything else start=False relies on
    per-element overwrite/accumulate.
  - tail split into two h-halves; next step's Wh rounds start as soon as
    the first half of h_t exists.  Cell chain is 4 fused STT ops + tanh.
  - 8 steps per hardware-loop body; a short fat-matmul warmup burst per
    body keeps the PE HAM clock at 2.4 GHz.
  - F8: Wh/Wx stationaries quantized to fp8e4 (x S), dequant folded into
    the gate activation scale (1/S).
"""

import math
import sys

sys.path.insert(0, "/root/shim")
sys.path.insert(0, "/opt/trn_rl_repo")

import numpy as np
import ml_dtypes

try:
    import antenv

    if "/root/shim/antenv" not in list(antenv.__path__):
        antenv.__path__.append("/root/shim/antenv")
except Exception:
    pass

import concourse.bass as bass
import concourse.bacc as bacc
import concourse.mybir as mybir
from concourse.tile import TileContext
from concourse.bass_utils import run_bass_kernel_spmd

FP32 = mybir.dt.float32
BF16 = mybir.dt.bfloat16
F8E4 = mybir.dt.float8e4

# Problem constants (hardcoded per harness contract)
N, T, D, H = 64, 128, 512, 1024
NC = 8            # cores
NL = N // NC      # samples per core = 8
G = 4 * H         # 4096 gate columns
L = 16            # attention locations
HC = H // 128     # 8 h-chunks
GM = G // 128     # 32 gate-col chunks
DC = D // 128     # 4 d-chunks
INV_SQRT_H = 1.0 / math.sqrt(H)

F8 = False            # quantize Wh/Wx stationaries to fp8e4
UNROLL = 16           # steps per hardware-loop body
KG = 4                # timesteps per x@Wx group
WARM = 0              # fat warmup matmuls per body (0 = off)
GCHAIN = False        # cell chain on GpSimd instead of Vector
SQ = 1000.0 if F8 else 1.0   # fp8 quantization scale (compile-time)

ADD = mybir.AluOpType.add
MULT = mybir.AluOpType.mult
TANH = mybir.ActivationFunctionType.Tanh

W_DT = F8E4 if F8 else BF16
W_NP = ml_dtypes.float8_e4m3 if F8 else ml_dtypes.bfloat16


def build_nc(timesteps=T):
    nc = bacc.Bacc()

    # ---- DRAM I/O (host-prepped layouts) ----
    # padded by 2*KG steps so the in-loop group prefetch never reads OOB
    xT_d = nc.dram_tensor(
        "xT", [128, DC, NL, timesteps + 2 * KG], BF16, kind="ExternalInput"
    )
    afT_d = nc.dram_tensor("afT", [128, HC, NL, L], FP32, kind="ExternalInput")
    wx_d = nc.dram_tensor("wx", [128, DC, G], W_DT, kind="ExternalInput")
    wh_d = nc.dram_tensor("wh", [128, HC, G], W_DT, kind="ExternalInput")
    wattn_d = nc.dram_tensor("wattn", [128, HC, G], BF16, kind="ExternalInput")
    bm_d = nc.dram_tensor("biasmat", [128, 128], BF16, kind="ExternalInput")
    ind_d = nc.dram_tensor("ind", [128, 2, GM // 2 * NL], BF16, kind="ExternalInput")
    mask_d = nc.dram_tensor("mask", [128, NL], FP32, kind="ExternalInput")
    bmask_d = nc.dram_tensor("bmask", [128, 128], BF16, kind="ExternalInput")
    out_d = nc.dram_tensor("hsT", [timesteps, 128, HC, NL], BF16, kind="ExternalOutput")

    with TileContext(nc) as tc:
        with tc.tile_pool(name="persist", bufs=1) as pp:
            afTb = pp.tile([128, HC, NL, L], BF16)     # Af^T bf16 scaled 1/256
            p_sb = pp.tile([128, G], BF16)             # P[(n,l), g] (x S)
            wh_sb = pp.tile([128, HC, G], W_DT)        # Wh tiles (x S)
            wx_sb = pp.tile([128, DC, G], W_DT)        # Wx tiles (x S)
            xgs = [
                pp.tile([128, KG, DC, NL], BF16, name=f"xg{i}") for i in range(2)
            ]                                          # staged x for a 4-step group
            bm_sb = pp.tile([128, 128], BF16)          # bias outer (x S)
            ind_sb = pp.tile([128, 2, GM // 2 * NL], BF16)  # indicator p==b*16+m
            mask_sb = pp.tile([128, NL], FP32)
            bmask_sb = pp.tile([128, 128], BF16)       # 16-block partition mask
            hTbs = [
                pp.tile([128, HC, NL], BF16, name=f"hTb{i}") for i in range(2)
            ]                                          # double-buffered h^T (2h)
            cT = pp.tile([128, HC, NL], FP32)          # 2c convention

            nc.sync.dma_start(mask_sb[:], mask_d[:])
            nc.sync.dma_start(bmask_sb[:], bmask_d[:])
            nc.sync.dma_start(bm_sb[:], bm_d[:])
            nc.sync.dma_start(ind_sb[:], ind_d[:])
            nc.sync.dma_start(wh_sb[:], wh_d[:])
            nc.sync.dma_start(wx_sb[:], wx_d[:])
            for gi in range(2):
                for k in range(KG):
                    t = gi * KG + k
                    nc.sync.dma_start(
                        xgs[gi][:, k, :, :],
                        xT_d[:, :, :, t:t + 1].rearrange(
                            "p d n t -> p d (n t)"
                        ),
                    )

            # ---------- P = Af^T @ (Wattn*S)  (bf16, one-time) ----------
            with (
                tc.tile_pool(name="setup_a", bufs=1) as sap,
                tc.tile_pool(name="wsl", bufs=2) as wslp,
                tc.tile_pool(name="ppsum", bufs=1, space="PSUM") as ppp,
            ):
                afT = sap.tile([128, HC, NL, L], FP32)
                nc.sync.dma_start(afT[:], afT_d[:])
                # afTb scaled 1/256: s_col becomes u = (s/32)/4 directly
                nc.vector.tensor_scalar_mul(afTb[:], afT[:], 1.0 / 256.0)
                afTr = sap.tile([128, HC, NL, L], BF16)  # unscaled, for P
                nc.vector.tensor_copy(afTr[:], afT[:])

                # h0 = mean over l of Af; cT = 2*c0, hTb = 2*h0
                nc.vector.tensor_reduce(
                    cT[:], afT[:], axis=mybir.AxisListType.X,
                    op=mybir.AluOpType.add,
                )
                nc.vector.tensor_scalar_mul(cT[:], cT[:], 2.0 / L)
                nc.vector.tensor_copy(hTbs[0][:], cT[:])

                pps = [
                    ppp.tile([128, 1024], FP32, tag=f"pps{gc}", name=f"pps{gc}")
                    for gc in range(4)
                ]
                for hc in range(HC):
                    wsl = wslp.tile([128, G], BF16, tag="wsl")
                    nc.sync.dma_start(wsl[:], wattn_d[:, hc, :])
                    for gc in range(4):
                        for hf in range(2):
                            nc.tensor.matmul(
                                pps[gc][:, hf * 512:(hf + 1) * 512],
                                afTr[:, hc, :, :].rearrange("p n l -> p (n l)"),
                                wsl[
                                    :,
                                    gc * 1024 + hf * 512:gc * 1024 + (hf + 1) * 512,
                                ],
                                start=(hc == 0),
                                stop=(hc == HC - 1),
                            )
                for gc in range(4):
                    nc.vector.tensor_copy(
                        p_sb[:, gc * 1024:(gc + 1) * 1024], pps[gc][:]
                    )

            # ---------- recurrence ----------
            with (
                tc.tile_pool(name="step", bufs=2) as sp,
                tc.tile_pool(name="gpsum", bufs=1, space="PSUM") as gp,
                tc.tile_pool(name="spsum", bufs=1, space="PSUM") as ssp,
            ):
                # grouped gate PSUM, split by m-half so each tile is one
                # bank and k-major (contiguous per step -> no false deps)
                GH = GM // 2
                aTgs = [
                    [
                        gp.tile(
                            [128, KG, GH, NL], FP32,
                            tag=f"aTg{i}{b}", name=f"aTg{i}{b}",
                        )
                        for b in range(2)
                    ]
                    for i in range(2)
                ]
                scp = ssp.tile([128, NL], FP32, tag="scp", name="scp")
                zsp = ssp.tile([128, 1], FP32, tag="zsp", name="zsp")
                ascale = 1.0 / SQ  # gl ACT input scale (fp8 dequant)
                veng = nc.gpsimd if GCHAIN else nc.vector

                def mm(out, lhsT, rhs, start=False, stop=False):
                    nc.tensor.matmul(
                        out, lhsT, rhs,
                        start=start, stop=stop, skip_group_check=True,
                    )

                def xdc(gpar, dc):
                    """One x@Wx contraction round for a 4-step group
                    (32-wide moving).  No h dependency: fills PE idle."""
                    xg = xgs[gpar]
                    for m in range(GM):
                        mm(
                            aTgs[gpar][m // GH][:, :, m % GH, :],
                            wx_sb[:, dc, m * 128:(m + 1) * 128],
                            xg[:, :, dc, :],
                            # first write into each of the two banks
                            start=(dc == 0 and m in (0, GH)),
                        )

                def xprefetch(ti_first, gpar):
                    xg = xgs[gpar]
                    for k in range(KG):
                        nc.sync.dma_start(
                            xg[:, k, :, :],
                            xT_d[
                                :, :, :, bass.ds(ti_first + k, 1)
                            ].rearrange("p d n t -> p d (n t)"),
                        )

                def step(ti, kslot, gpar, par, hin, hout, xfill=None):
                    aA, aB = aTgs[gpar]
                    # bias: a[g,(m,n)] += B[b*16+m,g] via indicator moving
                    mm(aA[:, kslot, :, :], bm_sb[:], ind_sb[:, 0, :])
                    mm(aB[:, kslot, :, :], bm_sb[:], ind_sb[:, 1, :])

                    # ----- PE: Wh k-chunks -----
                    def wh_k(kcs):
                        for kc in kcs:
                            for m in range(GM):
                                mm(
                                    aTgs[gpar][m // GH][:, kslot, m % GH, :],
                                    wh_sb[:, kc, m * 128:(m + 1) * 128],
                                    hin[:, kc, :],
                                )

                    wh_k(range(0, 4))

                    # ----- PE: scores (softmax hides under Wh rounds) -----
                    for kc in range(HC):
                        nc.tensor.matmul(
                            scp[:],
                            afTb[:, kc, :, :].rearrange("p n l -> p (n l)"),
                            hin[:, kc, :],
                            start=(kc == 0),
                            stop=(kc == HC - 1),
                        )

                    # ----- V: diagonal-block extract s_col -----
                    junk = sp.tile([128, NL], FP32, tag="junk")
                    s_col = sp.tile([128, 1], FP32, tag="s_col")
                    nc.vector.scalar_tensor_tensor(
                        junk[:], scp[:], 1.0, mask_sb[:],
                        MULT, MULT,
                        accum_out=s_col[:],
                    )
                    # ----- V: e^x via deg-5 Taylor of e^u, x=4u, square twice
                    ecol_t = sp.tile([128, 1], FP32, tag="ecol_t")
                    u_ap = s_col[:, 0:1]
                    nc.vector.tensor_scalar(
                        ecol_t[:], s_col[:], 1.0 / 120.0, 1.0 / 24.0,
                        MULT, ADD,
                    )
                    for cc in (1.0 / 6.0, 0.5, 1.0, 1.0):
                        nc.vector.tensor_scalar(
                            ecol_t[:], ecol_t[:], u_ap, cc,
                            MULT, ADD,
                        )
                    e2 = sp.tile([128, 1], FP32, tag="e2")
                    nc.vector.tensor_scalar_mul(e2[:], ecol_t[:], ecol_t[:, 0:1])
                    e_col = sp.tile([128, 1], FP32, tag="e_col")
                    nc.vector.tensor_scalar_mul(e_col[:], e2[:], e2[:, 0:1])
                    e_colb = sp.tile([128, 1], BF16, tag="e_colb")
                    nc.vector.tensor_copy(e_colb[:], e_col[:])
                    # ----- PE: Wh k-chunks 4..7 (overlaps softmax) -----
                    wh_k(range(4, HC))

                    # h-independent PE filler runs while the softmax chain
                    # computes (zsp below waits on e_colb, folds on ee)
                    if xfill is not None:
                        xfill()

                    # ----- PE: per-sample sums replicated partition-major ----
                    nc.tensor.matmul(
                        zsp[:], bmask_sb[:], e_colb[:], start=True, stop=True
                    )
                    rz = sp.tile([128, 1], FP32, tag="rz")
                    nc.vector.reciprocal(rz[:], zsp[:])
                    # ee = (mask * e_col) * (1/z), fused two-AP tensor_scalar
                    ee = sp.tile([128, NL], BF16, tag="ee")
                    nc.vector.tensor_scalar(
                        ee[:], mask_sb[:], e_col[:, 0:1], rz[:, 0:1],
                        MULT, MULT,
                    )

                    # ----- PE: attention folds accumulate into aTg -----
                    for m in range(GM):
                        mm(
                            aTgs[gpar][m // GH][:, kslot, m % GH, :],
                            p_sb[:, m * 128:(m + 1) * 128],
                            ee[:],
                            stop=True,
                        )

                    # ----- tail: single block -----
                    # The group PSUM tile is one bank, so the next step's PE
                    # writes serialize behind ALL of this step's PSUM-reading
                    # activations (bank hazard).  Half-splitting buys nothing;
                    # one merged pass minimizes the ACT serialization.
                    # tile A holds quarters i (m0-7), f (m8-15); tile B o, g
                    aQA = aA[:, kslot, :, :].rearrange("p (q c) n -> p q c n", q=2)
                    aQB = aB[:, kslot, :, :].rearrange("p (q c) n -> p q c n", q=2)
                    glA = sp.tile([128, 2, HC, NL], FP32, tag="glA")
                    nc.scalar.activation(glA[:], aQA[:], TANH, scale=ascale)
                    glB = sp.tile([128, 2, HC, NL], FP32, tag="glB")
                    nc.scalar.activation(glB[:], aQB[:], TANH, scale=ascale)
                    gi = glA[:, 0]
                    gf = glA[:, 1]
                    go = glB[:, 0]
                    gg = glB[:, 1]
                    cs = cT[:, :, :]
                    # u = (gl_f + 1) * cT(2c) = 4 f c
                    u = sp.tile([128, HC, NL], FP32, tag="u")
                    veng.scalar_tensor_tensor(u[:], gf, 1.0, cs, ADD, MULT)
                    # t2 = (gl_i + 1) * gl_g = 2 i g
                    t2 = sp.tile([128, HC, NL], FP32, tag="t2")
                    veng.scalar_tensor_tensor(t2[:], gi, 1.0, gg, ADD, MULT)
                    # cT = 0.5*u + t2 = 2 c_new
                    veng.scalar_tensor_tensor(cs, u[:], 0.5, t2[:], MULT, ADD)
                    # tct = tanh(0.5 * cT) = tanh(c)
                    tct = sp.tile([128, HC, NL], FP32, tag="tct")
                    nc.scalar.activation(tct[:], cs, TANH, scale=0.5)
                    # h2 = (gl_o + 1) * tct
                    veng.scalar_tensor_tensor(
                        hout[:, :, :], go, 1.0, tct[:], ADD, MULT
                    )
                    nc.sync.dma_start(
                        out_d[bass.ds(ti, 1), :, :, :].rearrange(
                            "t p c n -> p (t c) n"
                        ),
                        hout[:, :, :],
                    )

                # prologue: group 0's x rounds (xg0 loaded in setup)
                for dc in range(DC):
                    xdc(0, dc)

                with tc.For_i(0, timesteps, UNROLL, staggered_reset=True) as ti0:
                    for g in range(UNROLL // KG):
                        gpar = g % 2
                        for k in range(KG):
                            s = g * KG + k

                            def xfill(g=g, k=k):
                                # next group's x round k fills the ee wait
                                xdc((g + 1) % 2, k)
                                if k == KG - 1:
                                    # refill this parity's x buffer (group g+2)
                                    xprefetch(ti0 + (g + 2) * KG, gpar)

                            step(
                                ti0 + s, k, gpar, s % 2,
                                hTbs[s % 2], hTbs[(s + 1) % 2],
                                xfill=xfill,
                            )

    nc.finalize()
    return nc


def prep_inputs(x, A, Wx, Wh, Wattn, b):
    """Host-side reshapes to device layouts; returns per-core input maps."""
    x = np.asarray(x, dtype=np.float32)
    A = np.asarray(A, dtype=np.float32)
    Wx = np.asarray(Wx, dtype=np.float32)
    Wh = np.asarray(Wh, dtype=np.float32)
    Wattn = np.asarray(Wattn, dtype=np.float32)
    b = np.asarray(b, dtype=np.float32)
    timesteps = x.shape[1]

    # weight layouts [p, kc, g] with k = kc*128 + p
    # per-gate-column scaling: i/f/o columns carry a 0.5 (tanh half-angle
    # trick), g columns stay full-scale; Wh gets an extra 0.5 (h2 = 2h).
    gsc = np.ones((G,), np.float32) * 0.5
    gsc[3 * H:] = 1.0
    whs = (0.5 * gsc) * Wh
    wxs = gsc * Wx
    S = SQ
    wh_h = np.ascontiguousarray(
        np.clip(S * whs, -240, 240)
        .reshape(HC, 128, G).transpose(1, 0, 2).astype(W_NP)
    )
    wx_h = np.ascontiguousarray(
        np.clip(S * wxs, -240, 240)
        .reshape(DC, 128, G).transpose(1, 0, 2).astype(W_NP)
    )
    wattn_h = np.ascontiguousarray(
        (S * gsc * Wattn).reshape(HC, 128, G).transpose(1, 0, 2).astype(
            ml_dtypes.bfloat16
        )
    )
    # bias outer: bm[p, g] = S*gsc*b[p*128+g] for p < GM
    bm_h = np.zeros((128, 128), np.float32)
    bm_h[:GM, :] = (S * gsc * b).reshape(GM, 128)
    bm_h = bm_h.astype(ml_dtypes.bfloat16)
    # indicator: ind[p, b, (m', n)] = (p == b*16 + m')
    GH = GM // 2
    ind_h = np.zeros((128, 2, GH, NL), np.float32)
    for b in range(2):
        for m in range(GH):
            ind_h[b * GH + m, b, m, :] = 1.0
    ind_h = ind_h.reshape(128, 2, GH * NL).astype(ml_dtypes.bfloat16)
    mask_h = np.zeros((128, NL), dtype=np.float32)
    for p in range(128):
        mask_h[p, p // L] = 1.0
    bmask_h = (
        np.arange(128)[:, None] // L == np.arange(128)[None, :] // L
    ).astype(ml_dtypes.bfloat16)

    in_maps = []
    for c in range(NC):
        xs = x[c * NL:(c + 1) * NL]          # (8, T, 512)
        As = A[c * NL:(c + 1) * NL].reshape(NL, H, L)  # (8, 1024, 16)
        # xT [p, dc, n, t] = x[n, t, dc*128+p], padded for group prefetch
        xT_h = np.zeros((128, DC, NL, timesteps + 2 * KG), np.float32)
        xT_h[:, :, :, :timesteps] = xs.reshape(
            NL, timesteps, DC, 128
        ).transpose(3, 2, 0, 1)
        xT_h = np.ascontiguousarray(xT_h.astype(ml_dtypes.bfloat16))
        # afT [p, hc, n, l] = Af[n, hc*128+p, l]
        afT_h = np.ascontiguousarray(
            As.reshape(NL, HC, 128, L).transpose(2, 1, 0, 3)
        )
        in_maps.append(
            {
                "xT": xT_h,
                "afT": afT_h,
                "wx": wx_h,
                "wh": wh_h,
                "wattn": wattn_h,
                "biasmat": bm_h,
                "ind": ind_h,
                "mask": mask_h,
                "bmask": bmask_h,
            }
        )
    return in_maps, S


_NC_CACHE = {}


def kernel(x, A, Wx, Wh, Wattn, b, trace=False):
    timesteps = x.shape[1]
    key = timesteps
    if key not in _NC_CACHE:
        _NC_CACHE[key] = build_nc(timesteps)
    nc = _NC_CACHE[key]
    in_maps, S = prep_inputs(x, A, Wx, Wh, Wattn, b)
    res = run_bass_kernel_spmd(nc, in_maps, list(range(NC)), trace=trace)
    outs = []
    for c in range(NC):
        hsT = res.results[c]["hsT"]  # (T, 128, HC, NL)
        # out[n, t, hc*128+p] = hsT[t, p, hc, n]
        outs.append(
            0.5
            * hsT.astype(np.float32).transpose(3, 0, 2, 1).reshape(
                NL, timesteps, H
            )
        )
    full = np.concatenate(outs, axis=0).astype(np.float32)
    kernel.last_result = res
    return full
